# revision 12
# baseline (speedup 1.0000x reference)
"""Multi-head attention (B=4, T=2048, D=1024, H=16, causal) on 8 trn2 NeuronCores.

Sharding: core c handles batch b = c//2 and head-group g = c%2 (8 heads,
512 model dims). Q/K/V projections are computed per-core for the core's
head slice (W_q/W_k/W_v column-sharded), attention runs fully on-core,
the output projection uses W_o row-sharded, and a pairwise AllReduce
combines the two partial outputs per batch.

All tensors stay fp32; matmuls run in float32r (full-rate fp32 PE mode).
Activations are kept transposed ([d, tokens]) on-chip so every matmul
operand is naturally K-major:
    Q^T = Wq^T.T @ X^T           (per 128-d' tile, accumulated in PSUM)
    S^T[k,q] = (K^T slice).T @ Q^T slice      (contraction d_k = 64)
    P^T = exp((S^T + causal_mask) / 8)
    [x^T | s] = V_aug.T @ P^T    (V augmented with a ones column -> row sums)
    x^T normalized by s via DVE reciprocal + PE outer-product replicate
    out^T partial = Wo^T.T @ x^T, + b_o/2, pairwise AllReduce.
"""
import os
import numpy as np
from contextlib import ExitStack

import concourse.bass as bass
import concourse.tile as tile
import concourse.mybir as mybir
from concourse.bass_utils import run_bass_kernel_spmd
from bass_rust import ScopedClock

f32 = mybir.dt.float32
f32r = mybir.dt.float32r
EXPF = mybir.ActivationFunctionType.Exp

B, T, D = 4, 2048, 1024
H, DK = 16, 64
N_CORES = 8
HPC = 8            # heads per core
DH = HPC * DK      # 512, model dims per core
NEG = -1.0e9

_MODE_MAP = {"sem-ge-imm": "sem-ge", "sem-eq-imm": "sem-eq", "sem-le-imm": "sem-le"}


def _patched_drain_and_barrier(self, tick_clock, wait_clock):
    # This walrus build rejects Drain/NoOp instructions that carry sync
    # waits ("Too many sync wait commands"), which the stock Tile tail
    # emits. Put the tail waits on individual EventSemaphore instructions
    # and use sem-only barriers instead of the drain butterfly.
    nc = self.nc
    collector = nc.sync.nop(nofuse=True, hint="tile_tail_wait")
    wait_clock.add_sem_waits(collector.ins, ScopedClock({None: tick_clock.global_clock}))
    si = collector.ins.sync_info
    waits = list(si.on_wait) if si else []
    if si:
        collector.ins.sync_info = mybir.SyncInfo(on_wait=[], on_update=[])
    assert self.sems is not None
    name2sem = {s.name: s for s in self.sems.allocated().values()}
    for w in waits:
        nc.sync.wait_op(name2sem[w.ant_name], w.wait_value, _MODE_MAP.get(w.wait_mode, "sem-ge"))
    nc.all_engine_barrier(sem_only=True)
    popped = nc._tile_sem_poison_stack.pop()
    assert popped is self._sem_poison
    nc.clear_and_free_semaphores(list(self.sems.allocated().values()))
    nc.all_engine_barrier(sem_only=True)


tile.TileContext._drain_and_barrier = _patched_drain_and_barrier


def _fixup_sync_waits(nc):
    """This walrus build accepts at most 1 sync wait per compute/DMA
    instruction (EventSemaphore: 2). Tile's add_semaphores can emit more.
    Hoist excess waits onto EventSemaphore instructions inserted just
    before the over-budget instruction on the same engine."""
    for bb in nc.main_func.blocks:
        insts = bb.instructions
        out = []
        changed = False
        for ins in insts:
            si = ins.sync_info
            cap = 2 if type(ins).__name__ == "InstEventSemaphore" else 1
            if si is not None and len(si.on_wait) > cap:
                waits = list(si.on_wait)
                keep, excess = waits[-1:], waits[:-1]
                for i in range(0, len(excess), 2):
                    ev = mybir.InstEventSemaphore(
                        name=nc.get_next_instruction_name(),
                        ins=[], outs=[],
                        sync_info=mybir.SyncInfo(on_wait=excess[i:i + 2], on_update=[]),
                    )
                    ev.engine = ins.engine
                    out.append(ev)
                ins.sync_info = mybir.SyncInfo(on_wait=keep, on_update=list(si.on_update))
                changed = True
            out.append(ins)
        if changed:
            bb.instructions = out


def _emit_kernel(nc):
    qT = nc.dram_tensor("qT", [D, T], f32r, kind="ExternalInput")
    kT = nc.dram_tensor("kT", [D, T], f32r, kind="ExternalInput")
    vT = nc.dram_tensor("vT", [D, T], f32r, kind="ExternalInput")
    wq = nc.dram_tensor("wqT", [D, DH], f32r, kind="ExternalInput")
    wk = nc.dram_tensor("wkT", [D, DH], f32r, kind="ExternalInput")
    wv = nc.dram_tensor("wvT", [D, DH], f32r, kind="ExternalInput")
    wo = nc.dram_tensor("woT", [DH, D], f32r, kind="ExternalInput")
    bq = nc.dram_tensor("bq", [128, 4], f32, kind="ExternalInput")
    bk = nc.dram_tensor("bk", [128, 4], f32, kind="ExternalInput")
    bv = nc.dram_tensor("bv", [128, DH], f32, kind="ExternalInput")
    bo = nc.dram_tensor("bo", [128, 8], f32, kind="ExternalInput")
    msk = nc.dram_tensor("msk", [4, 128, 512], f32, kind="ExternalInput")
    outT = nc.dram_tensor("outT", [D, T], f32, kind="ExternalOutput")

    with tile.TileContext(nc, num_cores=N_CORES) as tc, ExitStack() as ctx:
        const = ctx.enter_context(tc.tile_pool(name="const", bufs=1))
        perm = ctx.enter_context(tc.tile_pool(name="perm", bufs=1))
        dram = ctx.enter_context(tc.tile_pool(name="dram", bufs=1, space="DRAM"))

        # Persistent on-chip tensors: [p, i, t] = full[i*128+p, t]
        QT = perm.tile([128, 4, T], f32r)
        KT = perm.tile([128, 4, T], f32r)
        Vg = perm.tile([128, 16, HPC * 65], f32r)   # V_aug: per k-tile, 8 heads x (64 vals + 1 one)
        xT = perm.tile([128, 4, T], f32r)

        bq_t = const.tile([128, 4], f32)
        bk_t = const.tile([128, 4], f32)
        bv_t = const.tile([128, DH], f32)
        bo_t = const.tile([128, 8], f32)
        ones_t = const.tile([65, 64], f32r)
        nc.sync.dma_start(bq_t[:], bq[:])
        nc.sync.dma_start(bk_t[:], bk[:])
        nc.sync.dma_start(bv_t[:], bv[:])
        nc.sync.dma_start(bo_t[:], bo[:])
        nc.vector.memset(ones_t[:].bitcast(f32), 1.0)
        # ones column of V_aug, written once (columns 64 + 65*n, uniform stride)
        nc.vector.memset(Vg[:].rearrange("p i (h j) -> p (i h) j", j=65)[:, :, 64:65].bitcast(f32), 1.0)

        cc_in = dram.tile([D, T], f32)
        cc_out = dram.tile([D, T], f32)

        # ---------------- Q / K projections ----------------
        with ExitStack() as ph:
            wpool = ph.enter_context(tc.tile_pool(name="wproj", bufs=2))
            xpool = ph.enter_context(tc.tile_pool(name="xchunk", bufs=2))
            qk_psum = ph.enter_context(tc.tile_pool(name="qk_psum", bufs=2, space="PSUM"))
            v_psum = ph.enter_context(tc.tile_pool(name="v_psum", bufs=2, space="PSUM"))

            for name, wdram, xdram, dst, bias in (
                ("q", wq, qT, QT, bq_t),
                ("k", wk, kT, KT, bk_t),
            ):
                wt = wpool.tile([128, 8, DH], f32r, tag="wproj")
                nc.sync.dma_start(wt[:], wdram.rearrange("(i p) n -> p i n", p=128))
                xsrc = xdram.rearrange("(i p) t -> p i t", p=128)
                for tck in range(4):
                    xc = xpool.tile([128, 8, 512], f32r, tag="xchunk")
                    nc.sync.dma_start(xc[:], xsrc[:, :, tck * 512:(tck + 1) * 512])
                    for e in range(4):
                        ps = qk_psum.tile([128, 512], f32)
                        for kt in range(8):
                            nc.tensor.matmul(
                                ps[:],
                                wt[:, kt, e * 128:(e + 1) * 128],
                                xc[:, kt, :],
                                start=(kt == 0), stop=(kt == 7),
                            )
                        nc.vector.tensor_add(
                            dst[:, e, tck * 512:(tck + 1) * 512], ps[:],
                            bias[:, e:e + 1].to_broadcast((128, 512)),
                        )

            # ---------------- V projection (natural layout, into V_aug) ----------------
            wvt = wpool.tile([128, 8, DH], f32r, tag="wproj")
            nc.sync.dma_start(wvt[:], wv.rearrange("(i p) n -> p i n", p=128))
            vsrc = vT.rearrange("(i p) t -> p i t", p=128)
            bv3 = bv_t[:].rearrange("p (h j) -> p h j", h=HPC)
            for tg in range(4):
                xc = xpool.tile([128, 8, 512], f32r, tag="xchunk")
                nc.sync.dma_start(xc[:], vsrc[:, :, tg * 512:(tg + 1) * 512])
                for tt in range(4):
                    ps = v_psum.tile([128, DH], f32)
                    for kt in range(8):
                        nc.tensor.matmul(
                            ps[:],
                            xc[:, kt, tt * 128:(tt + 1) * 128],
                            wvt[:, kt, :],
                            start=(kt == 0), stop=(kt == 7),
                        )
                    ti = tg * 4 + tt
                    nc.vector.tensor_add(
                        Vg[:, ti, :].rearrange("p (h j) -> p h j", h=HPC)[:, :, 0:64],
                        ps[:].rearrange("p (h j) -> p h j", h=HPC),
                        bv3,
                    )

        # ---------------- attention ----------------
        with ExitStack() as ph:
            mpool = ph.enter_context(tc.tile_pool(name="mpool", bufs=1))
            ppool = ph.enter_context(tc.tile_pool(name="ppool", bufs=20))
            rpool = ph.enter_context(tc.tile_pool(name="rpool", bufs=2))
            s_psum = ph.enter_context(tc.tile_pool(name="s_psum", bufs=3, space="PSUM"))
            pv_psum = ph.enter_context(tc.tile_pool(name="pv_psum", bufs=2, space="PSUM"))
            rep_psum = ph.enter_context(tc.tile_pool(name="rep_psum", bufs=2, space="PSUM"))

            mt = mpool.tile([128, 4, 512], f32)
            nc.sync.dma_start(mt[:], msk.rearrange("j p n -> p j n"))

            for h in range(HPC):
                po = 64 * (h % 2)
                hi = h // 2
                for qc in range(4):
                    q0 = qc * 512
                    nkt = 4 * qc + 4
                    ptiles = []
                    for kt in range(nkt):
                        sp = s_psum.tile([128, 512], f32)
                        nc.tensor.matmul(
                            sp[:],
                            KT[po:po + 64, hi, kt * 128:(kt + 1) * 128],
                            QT[po:po + 64, hi, q0:q0 + 512],
                        )
                        j = kt - 4 * qc
                        if j >= 0:
                            w = j * 128 + 128
                            nc.vector.tensor_add(sp[:, 0:w], sp[:, 0:w], mt[:, j, 0:w])
                        pt = ppool.tile([128, 512], f32r, tag="ptile")
                        nc.scalar.activation(pt[:], sp[:], EXPF, scale=0.125)
                        ptiles.append(pt)
                    pv = pv_psum.tile([65, 512], f32)
                    for kt in range(nkt):
                        nc.tensor.matmul(
                            pv[:],
                            Vg[:, kt, 65 * h:65 * (h + 1)],
                            ptiles[kt][:],
                            start=(kt == 0), stop=(kt == nkt - 1),
                        )
                    # normalize: r = 1/rowsum (lives on partition 64), replicate to
                    # 64 partitions via ones outer product, multiply into x^T
                    rr = rpool.tile([65, 512], f32r, tag="rrow")
                    with nc.allow_low_precision(reason="softmax denom reciprocal in f32r"):
                        nc.vector.reciprocal(rr[64:65, :], pv[64:65, :])
                    rp = rep_psum.tile([64, 512], f32)
                    nc.tensor.matmul(
                        rp[:], ones_t[64:65, :], rr[64:65, :]
                    )
                    nc.vector.tensor_copy(xT[po:po + 64, hi, q0:q0 + 512], pv[0:64, :])
                    nc.vector.tensor_mul(
                        xT[po:po + 64, hi, q0:q0 + 512],
                        xT[po:po + 64, hi, q0:q0 + 512],
                        rp[:],
                    )

        # ---------------- output projection + pairwise AllReduce ----------------
        with ExitStack() as ph:
            wopool = ph.enter_context(tc.tile_pool(name="wopool", bufs=1))
            opool = ph.enter_context(tc.tile_pool(name="opool", bufs=3))
            o_psum = ph.enter_context(tc.tile_pool(name="o_psum", bufs=3, space="PSUM"))

            wot = wopool.tile([128, 4, D], f32r)
            nc.sync.dma_start(wot[:], wo.rearrange("(i p) n -> p i n", p=128))

            for e in range(8):
                for tck in range(4):
                    ps = o_psum.tile([128, 512], f32)
                    for kt in range(4):
                        nc.tensor.matmul(
                            ps[:],
                            wot[:, kt, e * 128:(e + 1) * 128],
                            xT[:, kt, tck * 512:(tck + 1) * 512],
                            start=(kt == 0), stop=(kt == 3),
                        )
                    ot = opool.tile([128, 512], f32, tag="otile")
                    nc.vector.tensor_add(
                        ot[:], ps[:], bo_t[:, e:e + 1].to_broadcast((128, 512))
                    )
                    nc.sync.dma_start(
                        cc_in[e * 128:(e + 1) * 128, tck * 512:(tck + 1) * 512], ot[:]
                    )

            nc.gpsimd.collective_compute(
                "AllReduce",
                mybir.AluOpType.add,
                replica_groups=[[0, 1], [2, 3], [4, 5], [6, 7]],
                ins=[cc_in[:].opt()],
                outs=[cc_out[:].opt()],
            )
            nc.sync.dma_start(outT[:], cc_out[:])


_NC_CACHE = None


def _build_nc():
    global _NC_CACHE
    if _NC_CACHE is None:
        nc = bass.Bass("TRN2", target_bir_lowering=False, debug=False, num_devices=N_CORES)
        _emit_kernel(nc)
        _fixup_sync_waits(nc)
        _NC_CACHE = nc
    return _NC_CACHE


def _host_mask_tiles(attention_mask, key_padding_mask):
    # The kernel exploits the causal structure; verify the runtime masks
    # actually match it (they do for this problem's setup_inputs()).
    am = np.asarray(attention_mask)[0]
    causal = np.triu(np.ones((T, T), np.int32), k=1)
    if not np.array_equal(am != 0, causal != 0):
        raise ValueError("kernel specialised for strict-upper-triangular causal mask")
    if np.asarray(key_padding_mask).any():
        raise ValueError("kernel specialised for all-attendable key_padding_mask")
    # masks[j][dk, n] for the diagonal-region k-tile at column offset j*128 of a
    # 512-wide S^T chunk: S^T[k0+dk, q0+n] is masked iff (q0+n) < (k0+dk),
    # where k0 - q0 = j*128  ->  masked iff n < j*128 + dk.
    m = np.zeros((4, 128, 512), np.float32)
    for j in range(4):
        dk = np.arange(128)[:, None]
        n = np.arange(512)[None, :]
        m[j] = np.where(n < j * 128 + dk, NEG, 0.0)
    return m


def _make_in_maps(inputs):
    query = np.ascontiguousarray(np.asarray(inputs["query"], np.float32))
    key = np.ascontiguousarray(np.asarray(inputs["key"], np.float32))
    value = np.ascontiguousarray(np.asarray(inputs["value"], np.float32))
    W = {n: np.asarray(inputs[n], np.float32) for n in ("W_q", "W_k", "W_v", "W_o")}
    b = {n: np.asarray(inputs[n], np.float32) for n in ("b_q", "b_k", "b_v", "b_o")}
    msk = _host_mask_tiles(inputs["attention_mask"], inputs["key_padding_mask"])

    in_maps = []
    for c in range(N_CORES):
        bb, g = c // 2, c % 2
        hsel = slice(DH * g, DH * (g + 1))
        in_maps.append({
            "qT": np.ascontiguousarray(query[bb].T),
            "kT": np.ascontiguousarray(key[bb].T),
            "vT": np.ascontiguousarray(value[bb].T),
            "wqT": np.ascontiguousarray(W["W_q"].T[:, hsel]),
            "wkT": np.ascontiguousarray(W["W_k"].T[:, hsel]),
            "wvT": np.ascontiguousarray(W["W_v"].T[:, hsel]),
            "woT": np.ascontiguousarray(W["W_o"].T[hsel, :]),
            "bq": np.ascontiguousarray(b["b_q"][hsel].reshape(4, 128).T),
            "bk": np.ascontiguousarray(b["b_k"][hsel].reshape(4, 128).T),
            "bv": np.tile(b["b_v"][hsel][None, :], (128, 1)),
            "bo": np.ascontiguousarray((0.5 * b["b_o"]).reshape(8, 128).T),
            "msk": msk,
        })
    return in_maps


def kernel(**inputs):
    nc = _build_nc()
    in_maps = _make_in_maps(inputs)
    res = run_bass_kernel_spmd(nc, in_maps, core_ids=list(range(N_CORES)))
    out = np.empty((B, T, D), np.float32)
    for bb in range(B):
        out[bb] = res.results[2 * bb]["outT"].T
    return out
